# revision 1
# baseline (speedup 1.0000x reference)
"""AnyLoc VLAD (vq_codebook) Trainium2 kernel, 8-core data parallel.

Reference computation (per image, N=1024 patches, K=64 clusters, D=1536):
  descs_n = l2norm(query_descs)                 # row-normalize descriptors
  labels  = argmax_k(descs_n . l2norm(centers)) # hard assignment
  sum_d_k = sum_{n: label=k} descs_n            # per-cluster sum
  un_vlad = sum_d_k - count_k * centers_k
  vlad    = l2norm_rows(un_vlad); flatten; l2norm

Sharding: data-parallel over the batch axis, 4 images per NeuronCore; each
core holds the whole (tiny) codebook; host concatenates the per-core
[4, K*D] outputs (no collectives needed).

Device strategy (per core, fp8 matmul inputs):
  - host pre-casts descriptors to fp8e4m3 in BOTH layouts (natural [n,d]
    and pretiled-transposed [d,n] tiles) and ships the x64-scaled fp8
    codebook; argmax is invariant to each descriptor's own norm and to a
    uniform scale on sims, and un_vlad is scale-invariant under l2norm, so
    all x64/(1/64) factors cancel downstream
  - per 128-patch chunk: 12 accumulating fp8 TensorE matmuls produce sims
    [128,64], and 12 more reuse the same stationary weights for a Gram
    block whose diagonal IS the squared descriptor norms (identical fp8
    math; one DVE masked-reduce extracts it, no elementwise square pass);
    the gram matmuls never issue `start` so the sims group's pending-zero
    covers the shared PSUM bank; DVE row-max + one fused (sims>=max)*inv64
    op -> scaled one-hot assign [128,64]fp8; fp8 DoubleRow aggregation
    contracts chunk PAIRS (assign^T @ descs) into a per-image PSUM group
    [64,3x512] holding 64*sum_desc
  - per image: counts accumulate in one matmul burst at finalize (assign
    and norm tiles stay resident), so the per-image PSUM footprint is 3
    banks and double-buffers across images; -64*counts expands into a
    diagonal bf16 lhsT (identity-mask multiply) whose matmul vs centers
    ACCUMULATES into the same PSUM group, materializing 64*un_vlad in PSUM
    with no vector-engine pass; row norms + a ones-matmul broadcast of the
    global norm produce the final scale, which ACT applies reading straight
    from PSUM; DMA out

Toolchain workarounds: this walrus build accepts only one sync wait per
instruction, so Tile's tail drain is re-spread across per-engine drains
and a post-pass hoists surplus waits onto no-op carriers.
"""

import os
import sys

import numpy as np

for _p in ("/opt/trn_rl_repo", "/root/.axon_site/_ro/trn_rl_repo"):
    if os.path.isdir(_p) and _p not in sys.path:
        sys.path.insert(0, _p)

import ml_dtypes
import bass_rust
import concourse.bass as bass
import concourse.tile as tile
from concourse import mybir
from concourse.bass_utils import run_bass_kernel_spmd

B, N, K, D = 32, 1024, 64, 1536
NCORES = 8
IMGS = B // NCORES  # images per core
P = 128
NCH = N // P   # 8 patch chunks per image
DC = D // P    # 12 feature chunks
BF16 = mybir.dt.bfloat16
FP8 = mybir.dt.float8e4
F32 = mybir.dt.float32
F32R = mybir.dt.float32r
NP_BF16 = ml_dtypes.bfloat16
NP_FP8 = ml_dtypes.float8_e4m3
Alu = mybir.AluOpType
Act = mybir.ActivationFunctionType
EPS = 1e-12


def _patch_tile_drain():
    """This walrus build only accepts ONE sync wait per instruction; Tile's
    tail drain aggregates every outstanding semaphore wait onto a single
    Drain. Spread the waits across extra per-engine drains (all still
    before the end-of-kernel barrier, so semantics are unchanged)."""
    if getattr(tile.TileContext, "_vlad_drain_patched", False):
        return
    from concourse.vector_clock import ScopedClock

    def patched(self, tick_clock, wait_clock):
        nc = self.nc
        probe = nc.sync.drain()
        wait_clock.add_sem_waits(
            probe.ins, ScopedClock({None: tick_clock.global_clock})
        )
        si = probe.ins.sync_info
        waits = list(si.on_wait) if si is not None else []
        upds = list(si.on_update) if si is not None else []
        probe.ins.sync_info = bass_rust.SyncInfo(on_wait=waits[:1], on_update=upds)
        engines = [nc.scalar, nc.vector, nc.tensor, nc.gpsimd, nc.sync]
        for i, w in enumerate(waits[1:]):
            d = engines[i % len(engines)].drain()
            dsi = d.ins.sync_info
            du = list(dsi.on_update) if dsi is not None else []
            d.ins.sync_info = bass_rust.SyncInfo(on_wait=[w], on_update=du)
        nc.all_engine_barrier()
        popped = nc._tile_sem_poison_stack.pop()
        assert popped is self._sem_poison
        nc.clear_and_free_semaphores(list(self.sems.allocated().values()))

    tile.TileContext._drain_and_barrier = patched
    tile.TileContext._vlad_drain_patched = True


def _split_multi_waits(nc):
    """Walrus here accepts only one sync wait per instruction. Hoist surplus
    waits onto no-op carrier instructions inserted just before, on the same
    engine (safe: same engine executes in order, so all waits still complete
    before the original instruction issues)."""
    n_new = 0
    for _bbname, bassbb in list(nc.bb_map.items()):
        bb = bassbb.bb
        out = []
        changed = False
        for ins in bb.instructions:
            si = getattr(ins, "sync_info", None)
            waits = list(si.on_wait) if si is not None else []
            if len(waits) > 1:
                changed = True
                for w in waits[:-1]:
                    n_new += 1
                    nop = mybir.InstNoOp(
                        name=f"{ins.name}-wsplit{n_new}",
                        sync_info=mybir.SyncInfo(on_wait=[w], on_update=[]),
                        bass_nofuse=True,
                        engine=ins.engine,
                    )
                    nc.register_instruction(nop)
                    out.append(nop)
                ins.sync_info = bass_rust.SyncInfo(
                    on_wait=[waits[-1]], on_update=list(si.on_update)
                )
            out.append(ins)
        if changed:
            bb.instructions = out
    return n_new


def build_nc(imgs=IMGS, nch=NCH):
    """Build the per-core Bass graph. `imgs`/`nch` shrinkable for sim tests."""
    _patch_tile_drain()
    n_rows = imgs * nch * P
    nc = bass.Bass("TRN2", target_bir_lowering=False, debug=False)
    # fused per-chunk payload: row (b*nch+ci)*128+p holds the natural
    # descriptor row [0:D] followed by the pretiled-transposed row [D:2D]
    # (element (c, n) = descs[b, ci*128+n, c*128+p]), so each chunk needs a
    # single DMA instruction
    descs_e = nc.dram_tensor("descs", [n_rows, 2 * D], FP8, kind="ExternalInput")
    cnt_e = nc.dram_tensor("cnormt", [P, DC * K], FP8, kind="ExternalInput")
    cen_e = nc.dram_tensor("centers", [K, D], BF16, kind="ExternalInput")
    identm_e = nc.dram_tensor("identm", [P, K], F32, kind="ExternalInput")
    identg_e = nc.dram_tensor("identg", [P, P], F32, kind="ExternalInput")
    out_e = nc.dram_tensor("out", [imgs, K * D], F32, kind="ExternalOutput")

    with tile.TileContext(nc) as tc:
        from contextlib import ExitStack

        with ExitStack() as ctx:
            consts = ctx.enter_context(tc.tile_pool(name="consts", bufs=1))
            natp = ctx.enter_context(tc.tile_pool(name="nat", bufs=8))
            sqp = ctx.enter_context(tc.tile_pool(name="sq", bufs=5))
            smallp = ctx.enter_context(tc.tile_pool(name="small", bufs=20))
            # assigns and norms stay alive across a whole image: counts are
            # accumulated in one burst at finalize so the per-image psum
            # shrinks to 3 banks and double-buffers
            asnp = ctx.enter_context(tc.tile_pool(name="asn", bufs=2 * NCH + 2))
            nrmp = ctx.enter_context(tc.tile_pool(name="nrm", bufs=2 * NCH + 2))
            vladp = ctx.enter_context(tc.tile_pool(name="vlad", bufs=3))
            finp = ctx.enter_context(tc.tile_pool(name="fin", bufs=6))
            simsp = ctx.enter_context(tc.tile_pool(name="simsps", bufs=2, space="PSUM"))
            aggp = ctx.enter_context(tc.tile_pool(name="aggps", bufs=2, space="PSUM"))

            cnt_sb = consts.tile([P, DC, K], FP8)
            nc.sync.dma_start(
                out=cnt_sb, in_=cnt_e.ap().rearrange("p (c k) -> p c k", c=DC)
            )
            identg = consts.tile([P, P], F32)
            nc.sync.dma_start(out=identg, in_=identg_e.ap())
            # finalize-only consts are emitted lazily after image 0's chunk
            # stream so they load in its shadow instead of gating startup
            lateconsts = {}

            def _late_consts():
                if lateconsts:
                    return lateconsts
                cen_sb = consts.tile([K, D], BF16)
                nc.sync.dma_start(out=cen_sb, in_=cen_e.ap())
                ones_sb = consts.tile([P, K], BF16)
                nc.vector.memset(ones_sb, 1.0)
                # identm[p, j] = (j == p % 64): expands a per-partition
                # value into a diagonal matrix via one multiply
                identm = consts.tile([P, K], F32)
                nc.sync.dma_start(out=identm, in_=identm_e.ap())
                lateconsts.update(
                    cen_sb=cen_sb, ones_sb=ones_sb, identm=identm
                )
                return lateconsts

            for b in range(imgs):
                # per-image psum accumulator [64, 3, 512] fp32 = 64*sum_desc
                agg_ps = aggp.tile([K, 3, 512], F32)
                asns = []
                nrmqs = []
                for cp in range(nch // 2):
                    r0 = (b * nch + 2 * cp) * P
                    # pair tile with per-d 192-col combined regions
                    # [codebook 64 | transposed 128]; natural rows + the
                    # transposed blocks DMA from DRAM, the (constant)
                    # codebook blocks are filled SBUF->SBUF by idle GPSIMD,
                    # so one matmul per d computes sims+gram off one
                    # weight load with no extra HBM traffic
                    rowb = D + DC * (K + P)
                    pairt = natp.tile([P, 2, rowb], FP8)
                    nc.sync.dma_start(
                        out=pairt[:, :, 0:D],
                        in_=descs_e.ap()[r0 : r0 + 2 * P, 0:D].rearrange(
                            "(q p) d -> p q d", q=2
                        ),
                    )
                    for q in range(2):
                        combq = pairt[:, q, D:rowb].rearrange(
                            "p (c x) -> p c x", c=DC
                        )
                        nc.gpsimd.dma_start(
                            out=combq[:, :, K : K + P],
                            in_=descs_e.ap()[
                                r0 + q * P : r0 + (q + 1) * P, D : 2 * D
                            ].rearrange("p (c n) -> p c n", c=DC),
                        )
                        nc.gpsimd.dma_start(out=combq[:, :, 0:K], in_=cnt_sb[:])
                    asnpair = asnp.tile([P, 2, K], FP8)
                    nrmqpair = nrmp.tile([P, 2, 16], FP8, tag="nrmq")
                    asns.append(asnpair)
                    nrmqs.append(nrmqpair)
                    for q in range(2):
                        ci = 2 * cp + q
                        nat = pairt[:, q, 0:D]
                        comb = pairt[:, q, D:rowb].rearrange(
                            "p (c x) -> p c x", c=DC
                        )

                        # one matmul per d: out = tsp_d^T @ [cnt_d|tsp_d]
                        # -> sims in cols 0:K, gram in cols K:K+P; the gram
                        # diagonal is sum_d desc_d^2 on the same fp8 values
                        simsg = simsp.tile([P, K + P], F32, tag="sims")
                        sims = simsg[:, 0:K]
                        gram = simsg[:, K : K + P]
                        for d in range(DC):
                            nc.tensor.matmul(
                                simsg,
                                lhsT=comb[:, d, K : K + P],
                                rhs=comb[:, d, :],
                                start=(d == 0), stop=(d == DC - 1),
                            )
                        dummy = sqp.tile([P, P], BF16, tag="sq")
                        ss = smallp.tile([P, 1], F32, tag="ss")
                        nc.vector.scalar_tensor_tensor(
                            out=dummy, in0=gram, scalar=1.0, in1=identg,
                            op0=Alu.mult, op1=Alu.mult, accum_out=ss,
                        )
                        # nrmq = sqrt(ss)/64 in fp8; the /64 cancels against
                        # the x64 in inv downstream (scale-invariant)
                        nc.scalar.activation(
                            nrmqpair[:, q, 0:1], ss, Act.Sqrt,
                            scale=1.0 / 4096.0,
                        )
                        inv = smallp.tile([P, 1], F32, tag="inv")
                        nc.vector.reciprocal(inv, nrmqpair[:, q, 0:1])
                        mx = smallp.tile([P, 1], F32, tag="mx")
                        nc.vector.tensor_reduce(
                            mx, sims, axis=mybir.AxisListType.X, op=Alu.max
                        )
                        nc.vector.tensor_scalar(
                            asnpair[:, q, :], sims, scalar1=mx, scalar2=inv,
                            op0=Alu.is_ge, op1=Alu.mult,
                        )

                    # DoubleRow fp8 aggregation: both chunks of the pair in
                    # one matmul (virtual 256-row contraction, 2 fp8/cell)
                    first = cp == 0
                    for j in range(3):
                        nc.tensor.matmul(
                            agg_ps[:, j, :],
                            lhsT=asnpair,
                            rhs=pairt[:, :, j * 512 : (j + 1) * 512],
                            start=first, stop=False,
                            perf_mode=mybir.MatmulPerfMode.DoubleRow,
                            skip_group_check=True,
                        )

                # ---- finalize image b (emission deferred into the next
                # image's chunk stream so chunk ops keep scheduler priority;
                # executes concurrently thanks to the double-buffered agg) ----
                def _finalize(b=b, agg_ps=agg_ps, asns=asns, nrmqs=nrmqs):
                    lc = _late_consts()
                    _emit_finalize(
                        nc, tc, b, agg_ps, asns, nrmqs, nch, lc["cen_sb"],
                        lc["identm"], lc["ones_sb"], simsp, finp, sqp, vladp,
                        out_e,
                    )

                _finalize()


    _split_multi_waits(nc)
    return nc


def _emit_finalize(
    nc, tc, b, agg_ps, asns, nrmqs, nch, cen_sb, identm, ones_sb,
    simsp, finp, sqp, vladp, out_e,
):
    # counts in one burst (borrows a sims-pool bank briefly)
    counts_ps = simsp.tile([P, K], F32, tag="sims")
    npair = nch // 2
    for cp in range(npair):
        nc.tensor.matmul(
            counts_ps[0:K, 0:1], lhsT=asns[cp], rhs=nrmqs[cp][:, :, 0:1],
            start=(cp == 0), stop=(cp == npair - 1),
            perf_mode=mybir.MatmulPerfMode.DoubleRow,
            skip_group_check=True,
        )
    negc = finp.tile([K, 1], F32, tag="negc")
    nc.vector.tensor_scalar_mul(negc, counts_ps[0:K, 0:1], -64.0)
    # diagonal lhsT holding -64*counts; fp32r matmul vs centers
    # accumulates -64*counts_k*centers[k,:] into the same psum
    # group, materializing 64*un_vlad in PSUM
    diag = finp.tile([K, K], BF16, tag="diag")
    nc.vector.tensor_scalar(
        diag, identm[0:K], scalar1=negc, scalar2=None,
        op0=Alu.mult,
    )
    for j in range(3):
        nc.tensor.matmul(
            agg_ps[:, j, :],
            lhsT=diag,
            rhs=cen_sb[:, j * 512 : (j + 1) * 512],
            start=False, stop=(j == 2),
            skip_group_check=True,
        )
    uv = agg_ps[:, 0:3, :]
    sq2 = sqp.tile([P, D], FP8, tag="sq")
    r2 = finp.tile([K, 1], F32, tag="r2")
    nc.scalar.activation(sq2[0:K], uv, Act.Square, accum_out=r2)
    u = finp.tile([K, 1], F32, tag="u")
    nc.scalar.sqrt(u, r2)
    um = finp.tile([K, 1], F32, tag="um")
    nc.vector.tensor_scalar_max(um, u, EPS)
    invu = finp.tile([K, 1], F32, tag="invu")
    nc.vector.reciprocal(invu, um)
    s = finp.tile([K, 1], BF16, tag="s")
    nc.vector.tensor_scalar(
        s, u, scalar1=1e30, scalar2=1.0,
        op0=Alu.mult, op1=Alu.min,
    )
    # ones-matmul broadcasts G = sum_k s_k to every partition;
    # borrows a sims-pool slot for one bank
    g_ps = simsp.tile([P, K], F32, tag="sims")
    nc.tensor.matmul(
        g_ps[0:K, 0:1], lhsT=ones_sb[0:K], rhs=s,
        start=True, stop=True, skip_group_check=True,
    )
    sg = finp.tile([K, 1], F32, tag="sg")
    nc.scalar.sqrt(sg, g_ps[0:K, 0:1])
    ginv = finp.tile([K, 1], F32, tag="ginv")
    nc.vector.reciprocal(ginv, sg)
    tot = finp.tile([K, 1], F32, tag="tot")
    nc.vector.tensor_mul(tot, invu, ginv)
    vfin = vladp.tile([K, D], F32, tag="vfin")
    out_kd = out_e.ap()[b].rearrange("(k d) -> k d", k=K)
    for j in range(3):
        js = slice(j * 512, (j + 1) * 512)
        nc.scalar.mul(vfin[:, js], agg_ps[:, j, :], tot)
        nc.sync.dma_start(out=out_kd[:, js], in_=vfin[:, js])


def prep_inputs(query_descs, c_centers):
    """Host-side layout prep shared by kernel() and tests."""
    qd = np.ascontiguousarray(query_descs, dtype=np.float32)
    cc = np.ascontiguousarray(c_centers, dtype=np.float32)
    descs16 = qd.astype(NP_FP8)  # [B, N, D]
    cn = cc / np.maximum(np.linalg.norm(cc, axis=1, keepdims=True), EPS)
    # x64 so the fp8 codebook lands in e4m3's sweet spot; argmax and the
    # max-compare are invariant to a uniform positive scale on sims
    # packed so each partition's [DC, K] block is one contiguous DMA read
    cnt16 = np.ascontiguousarray(
        (cn.T * 64.0).astype(NP_FP8).reshape(DC, P, K).transpose(1, 0, 2)
    ).reshape(P, DC * K)
    identm = np.ascontiguousarray(
        np.tile(np.eye(K, dtype=np.float32), (P // K, 1))
    ).reshape(P, K)
    in_maps = []
    for core in range(NCORES):
        sh = descs16[core * IMGS : (core + 1) * IMGS]  # [IMGS, N, D]
        shard = sh.reshape(IMGS * N, D)
        # pretiled transpose: row (b*NCH+ci)*128+p holds [DC, 128n] with
        # element (c, n) = descs[b, ci*128+n, c*128+p]
        sht = np.ascontiguousarray(
            sh.reshape(IMGS, NCH, P, DC, P).transpose(0, 1, 4, 3, 2)
        ).reshape(IMGS * N, D)
        fused = np.ascontiguousarray(
            np.concatenate([shard, sht], axis=1)
        )  # [IMGS*N, 2D]
        in_maps.append(
            {
                "descs": fused,
                "cnormt": cnt16,
                "centers": cc.astype(NP_BF16),
                "identm": identm,
                "identg": np.eye(P, dtype=np.float32),
            }
        )
    return in_maps


_NC_CACHE = {}


def _get_nc():
    if "nc" not in _NC_CACHE:
        _NC_CACHE["nc"] = build_nc()
    return _NC_CACHE["nc"]


def kernel(query_descs, c_centers):
    in_maps = prep_inputs(query_descs, c_centers)
    nc = _get_nc()
    res = run_bass_kernel_spmd(nc, in_maps, core_ids=list(range(NCORES)))
    out = np.concatenate(
        [res.results[i]["out"] for i in range(NCORES)], axis=0
    )  # [B, K*D]
    return out.astype(np.float32)



# revision 2
# speedup vs baseline: 1.7201x; 1.7201x over previous
"""AnyLoc VLAD (vq_codebook) Trainium2 kernel, 8-core data parallel. v2.

Reference computation (per image, N=1024 patches, K=64 clusters, D=1536):
  descs_n = l2norm(query_descs)                 # row-normalize descriptors
  labels  = argmax_k(descs_n . l2norm(centers)) # hard assignment
  sum_d_k = sum_{n: label=k} descs_n            # per-cluster sum
  un_vlad = sum_d_k - count_k * centers_k
  vlad    = l2norm_rows(un_vlad); flatten; l2norm

Sharding: data-parallel over the batch axis, 4 images per NeuronCore; each
core holds the whole (tiny) codebook; host concatenates the per-core
[4, K*D] outputs (no collectives needed).

Device strategy (per core), redesigned from the v1 fused-gram kernel after
trace analysis showed (a) 100us of gpsimd *software* DMA from the
interleaved [codebook|transposed] SBUF layout, (b) ~200ns fixed cost per
matmul instruction making the 384 small sims matmuls dominate TensorE, and
(c) only sync+scalar engines have hardware DGE:

  - host pre-normalizes descriptors (fp8 cast at x64 scale) and ships TWO
    clean contiguous fp8 layouts: natural pair tiles (agg rhs) and a
    DoubleRow-packed transposed layout (sims rhs). All DMAs are big
    contiguous hardware-DGE transfers on the sync/scalar queues.
  - sims^T in codebook-stationary orientation: per image just 12 DoubleRow
    fp8 matmuls (contract 256 d-rows/pass, 512-col blocks) accumulate
    simsT [64, 1024] in PSUM. 32x fewer LDWEIGHTS than v1.
  - simsT -> bf16 SBUF copy (ACT) -> 8 PE transposes back to patch-major
    [128, 8, 64] PSUM (one bank, start only on the first: later transposes
    must not re-mark the zero-region or a HW lazy-zero read would wipe
    earlier chunks).
  - assignment: one segmented DVE row-max ([128,8,64] -> [128,8]) and one
    scalar_tensor_tensor is_ge against a stride-0 broadcast of the max:
    exact 1.0 one-hot in fp8 (2 DVE ops/image vs 96 in v1).
  - agg: 12 DoubleRow matmuls (assign pairs as stationary) + 4 one-column
    counts matmuls vs a ones tile: counts are exact integers.
  - finalize without the v1 diag-matmul: un_vlad = (-64*counts)*centers +
    agg in ONE DVE scalar_tensor_tensor reading the agg PSUM (frees the
    bank early); row norms via ACT Square+accum; global norm = sqrt(#
    nonzero rows) via a ones-matmul broadcast; ACT applies the final scale.
  - per-image work is two-stage software-pipelined: image b's agg/finalize
    ops are emitted inside image b+1's stream so TensorE alternates
    sims_b+1 / agg_b with no idle window; PSUM budget is exactly 8 banks
    (2 sims + 2x1 transpose + 1 counts + 3 agg).

Toolchain workarounds: this walrus build accepts only one sync wait per
instruction, so Tile's tail drain is re-spread across per-engine drains
and a post-pass hoists surplus waits onto no-op carriers.
"""

import os
import sys

import numpy as np

for _p in ("/opt/trn_rl_repo", "/root/.axon_site/_ro/trn_rl_repo"):
    if os.path.isdir(_p) and _p not in sys.path:
        sys.path.insert(0, _p)

from contextlib import ExitStack

import ml_dtypes
import bass_rust
import concourse.bass as bass
import concourse.tile as tile
from concourse import mybir
from concourse.bass_utils import run_bass_kernel_spmd

B, N, K, D = 32, 1024, 64, 1536
NCORES = 8
IMGS = B // NCORES  # images per core
P = 128
NPAIR = 4   # patch chunk-pairs per image (N = NPAIR*256)
CP = 6      # feature chunk-pairs (D = CP*256)
JJ = D // 512  # agg column blocks
BF16 = mybir.dt.bfloat16
FP8 = mybir.dt.float8e4
F32 = mybir.dt.float32
NP_BF16 = ml_dtypes.bfloat16
NP_FP8 = ml_dtypes.float8_e4m3
Alu = mybir.AluOpType
Act = mybir.ActivationFunctionType
DR = mybir.MatmulPerfMode.DoubleRow
EPS = 1e-12


def _patch_tile_drain():
    """This walrus build only accepts ONE sync wait per instruction; Tile's
    tail drain aggregates every outstanding semaphore wait onto a single
    Drain. Spread the waits across extra per-engine drains (all still
    before the end-of-kernel barrier, so semantics are unchanged)."""
    if getattr(tile.TileContext, "_vlad_drain_patched", False):
        return
    from concourse.vector_clock import ScopedClock

    def patched(self, tick_clock, wait_clock):
        nc = self.nc
        probe = nc.sync.drain()
        wait_clock.add_sem_waits(
            probe.ins, ScopedClock({None: tick_clock.global_clock})
        )
        si = probe.ins.sync_info
        waits = list(si.on_wait) if si is not None else []
        upds = list(si.on_update) if si is not None else []
        probe.ins.sync_info = bass_rust.SyncInfo(on_wait=waits[:1], on_update=upds)
        engines = [nc.scalar, nc.vector, nc.tensor, nc.gpsimd, nc.sync]
        for i, w in enumerate(waits[1:]):
            d = engines[i % len(engines)].drain()
            dsi = d.ins.sync_info
            du = list(dsi.on_update) if dsi is not None else []
            d.ins.sync_info = bass_rust.SyncInfo(on_wait=[w], on_update=du)
        nc.all_engine_barrier()
        popped = nc._tile_sem_poison_stack.pop()
        assert popped is self._sem_poison
        nc.clear_and_free_semaphores(list(self.sems.allocated().values()))

    tile.TileContext._drain_and_barrier = patched
    tile.TileContext._vlad_drain_patched = True


def _split_multi_waits(nc):
    """Walrus here accepts only one sync wait per instruction. Hoist surplus
    waits onto no-op carrier instructions inserted just before, on the same
    engine (safe: same engine executes in order, so all waits still complete
    before the original instruction issues)."""
    n_new = 0
    for _bbname, bassbb in list(nc.bb_map.items()):
        bb = bassbb.bb
        out = []
        changed = False
        for ins in bb.instructions:
            si = getattr(ins, "sync_info", None)
            waits = list(si.on_wait) if si is not None else []
            if len(waits) > 1:
                changed = True
                for w in waits[:-1]:
                    n_new += 1
                    nop = mybir.InstNoOp(
                        name=f"{ins.name}-wsplit{n_new}",
                        sync_info=mybir.SyncInfo(on_wait=[w], on_update=[]),
                        bass_nofuse=True,
                        engine=ins.engine,
                    )
                    nc.register_instruction(nop)
                    out.append(nop)
                ins.sync_info = bass_rust.SyncInfo(
                    on_wait=[waits[-1]], on_update=list(si.on_update)
                )
            out.append(ins)
        if changed:
            bb.instructions = out
    return n_new


def build_nc(imgs=IMGS, npair=NPAIR):
    """Build the per-core Bass graph. `imgs`/`npair` shrinkable for sim."""
    _patch_tile_drain()
    NN = npair * 2 * P        # patches per image
    nch = 2 * npair           # 128-patch chunks per image
    NB = (NN + 511) // 512    # sims column blocks
    nblocks = [(j * 512, min(512, NN - j * 512)) for j in range(NB)]

    nc = bass.Bass("TRN2", target_bir_lowering=False, debug=False)
    # natural pair tiles: row (b*npair+cp)*128+p = desc[(2cp)*128+p] ++
    # desc[(2cp+1)*128+p]  (3072 contiguous bytes)
    descsn_e = nc.dram_tensor("descsn", [imgs * npair * P, 2 * D], FP8,
                              kind="ExternalInput")
    # DoubleRow-packed transpose: row (b*CP+c)*128+p holds [q, n] with
    # element (q, n) = desc[b, n, 256c+128q+p]  (2*NN contiguous bytes)
    descst_e = nc.dram_tensor("descst", [imgs * CP * P, 2 * NN], FP8,
                              kind="ExternalInput")
    # codebook, same DoubleRow packing: row (c, p) = [q, k] = cnorm64[k, 256c+128q+p]
    cnt2_e = nc.dram_tensor("cnt2", [CP * P, 2 * K], FP8, kind="ExternalInput")
    cen_e = nc.dram_tensor("cen", [K, D], BF16, kind="ExternalInput")
    ident_e = nc.dram_tensor("ident", [K, K], BF16, kind="ExternalInput")
    out_e = nc.dram_tensor("out", [imgs, K * D], F32, kind="ExternalOutput")

    with tile.TileContext(nc) as tc:
        with ExitStack() as ctx:
            consts = ctx.enter_context(tc.tile_pool(name="consts", bufs=1))
            tspp = ctx.enter_context(tc.tile_pool(name="tspp", bufs=2))
            natp = ctx.enter_context(tc.tile_pool(name="natp", bufs=2))
            simsbp = ctx.enter_context(tc.tile_pool(name="simsbp", bufs=2))
            asnp = ctx.enter_context(tc.tile_pool(name="asnp", bufs=2))
            mxp = ctx.enter_context(tc.tile_pool(name="mxp", bufs=2))
            uvp = ctx.enter_context(tc.tile_pool(name="uvp", bufs=2))
            sqp = ctx.enter_context(tc.tile_pool(name="sqp", bufs=2))
            vfinp = ctx.enter_context(tc.tile_pool(name="vfinp", bufs=2))
            finp = ctx.enter_context(tc.tile_pool(name="finp", bufs=16))
            simsps = ctx.enter_context(
                tc.tile_pool(name="simsps", bufs=1, space="PSUM"))
            transps = ctx.enter_context(
                tc.tile_pool(name="transps", bufs=2, space="PSUM"))
            cntps = ctx.enter_context(
                tc.tile_pool(name="cntps", bufs=1, space="PSUM"))
            aggps = ctx.enter_context(
                tc.tile_pool(name="aggps", bufs=1, space="PSUM"))

            cnt_sb = consts.tile([P, CP, 2, K], FP8)
            nc.sync.dma_start(
                out=cnt_sb,
                in_=cnt2_e.ap().rearrange("(c p) (q k) -> p c q k", c=CP, q=2),
            )
            ident_sb = consts.tile([K, K], BF16)
            nc.sync.dma_start(out=ident_sb, in_=ident_e.ap())
            onesc = consts.tile([P, 2, 1], FP8)
            nc.vector.memset(onesc, 1.0)
            # finalize-only consts load lazily in image 0's shadow
            late = {}

            def _late():
                if not late:
                    cen_sb = consts.tile([K, D], BF16)
                    nc.scalar.dma_start(out=cen_sb, in_=cen_e.ap())
                    onesg = consts.tile([K, K], BF16)
                    nc.vector.memset(onesg, 1.0)
                    late.update(cen_sb=cen_sb, onesg=onesg)
                return late

            def emit_agg_fin1(s):
                lc = _late()
                agg = aggps.tile([K, JJ, 512], F32, tag="agg")
                for cp in range(npair):
                    for jj in range(JJ):
                        nc.tensor.matmul(
                            agg[:, jj, :],
                            lhsT=s["asn"][:, 2 * cp:2 * cp + 2, :],
                            rhs=s["pairt"][:, cp, :, jj * 512:(jj + 1) * 512],
                            start=(cp == 0), stop=(cp == npair - 1),
                            perf_mode=DR, skip_group_check=True,
                        )
                counts = cntps.tile([K, 1], F32, tag="cnt")
                for cp in range(npair):
                    nc.tensor.matmul(
                        counts, lhsT=s["asn"][:, 2 * cp:2 * cp + 2, :],
                        rhs=onesc, start=(cp == 0), stop=(cp == npair - 1),
                        perf_mode=DR, skip_group_check=True,
                    )
                negc = finp.tile([K, 1], F32, tag="negc")
                nc.vector.tensor_scalar_mul(negc, counts, -64.0)
                # uv = (-64*counts)*centers + 64*sum_desc, read straight
                # from the agg psum (frees the banks early)
                uv = uvp.tile([K, D], F32, tag="uv")
                nc.vector.scalar_tensor_tensor(
                    out=uv.rearrange("k (a b) -> k a b", a=JJ),
                    in0=lc["cen_sb"].rearrange("k (a b) -> k a b", a=JJ),
                    scalar=negc, in1=agg, op0=Alu.mult, op1=Alu.add,
                )
                sq = sqp.tile([K, D], FP8, tag="sq")
                r2 = finp.tile([K, 1], F32, tag="r2")
                nc.scalar.activation(sq, uv, Act.Square, accum_out=r2)
                u = finp.tile([K, 1], F32, tag="u")
                nc.scalar.sqrt(u, r2)
                s.update(uv=uv, u=u)

            def emit_fin2(s):
                lc = _late()
                um = finp.tile([K, 1], F32, tag="um")
                nc.vector.tensor_scalar_max(um, s["u"], EPS)
                invu = finp.tile([K, 1], F32, tag="invu")
                nc.vector.reciprocal(invu, um)
                sgate = finp.tile([K, 1], BF16, tag="sgate")
                nc.vector.tensor_scalar(
                    sgate, s["u"], scalar1=1e30, scalar2=1.0,
                    op0=Alu.mult, op1=Alu.min,
                )
                g_ps = cntps.tile([K, 1], F32, tag="cnt")
                nc.tensor.matmul(g_ps, lhsT=lc["onesg"], rhs=sgate,
                                 start=True, stop=True, skip_group_check=True)
                sg = finp.tile([K, 1], F32, tag="sg")
                nc.scalar.sqrt(sg, g_ps)
                ginv = finp.tile([K, 1], F32, tag="ginv")
                nc.vector.reciprocal(ginv, sg)
                tot = finp.tile([K, 1], F32, tag="tot")
                nc.vector.tensor_mul(tot, invu, ginv)
                vfin = vfinp.tile([K, D], F32, tag="vfin")
                nc.scalar.mul(vfin, s["uv"], tot)
                out_kd = out_e.ap()[s["b"]].rearrange("(k d) -> k d", k=K)
                nc.scalar.dma_start(out=out_kd, in_=vfin)

            prev = None
            for b in range(imgs):
                tsp = tspp.tile([P, CP, 2, NN], FP8, tag="tsp")
                nc.sync.dma_start(
                    out=tsp,
                    in_=descst_e.ap()[b * CP * P:(b + 1) * CP * P, :]
                    .rearrange("(c p) (q n) -> p c q n", c=CP, q=2),
                )
                pairt = natp.tile([P, npair, 2, D], FP8, tag="nat")
                nc.scalar.dma_start(
                    out=pairt,
                    in_=descsn_e.ap()[b * npair * P:(b + 1) * npair * P, :]
                    .rearrange("(c p) (q d) -> p c q d", c=npair, q=2),
                )
                # sims^T: codebook-stationary DoubleRow accumulation
                simsT = simsps.tile([K, NB, 512], F32, tag="sims")
                for j, (n0, nn) in enumerate(nblocks):
                    for c in range(CP):
                        nc.tensor.matmul(
                            simsT[:, j, 0:nn], lhsT=cnt_sb[:, c],
                            rhs=tsp[:, c, :, n0:n0 + nn],
                            start=(c == 0), stop=(c == CP - 1),
                            perf_mode=DR, skip_group_check=True,
                        )
                # deferred agg+fin (part 1) of the previous image overlaps
                # this image's sims/copy on tensor/vector
                if prev is not None:
                    emit_agg_fin1(prev)
                simsSb = simsbp.tile([K, NB, 512], BF16, tag="simsb")
                for j, (n0, nn) in enumerate(nblocks):
                    nc.scalar.mul(simsSb[:, j, 0:nn], simsT[:, j, 0:nn], 1.0)
                # transpose back to patch-major; ONE bank, start only on the
                # first write (re-marking the zero region would let a lazy
                # hardware zero wipe earlier chunks)
                trT = transps.tile([P, nch, K], BF16, tag="tr")
                for ch in range(nch):
                    j, o = (ch * P) // 512, (ch * P) % 512
                    nc.tensor.matmul(
                        trT[:, ch, :], lhsT=simsSb[:, j, o:o + P],
                        rhs=ident_sb, is_transpose=True,
                        start=(ch == 0), stop=(ch == nch - 1),
                        skip_group_check=True,
                    )
                if prev is not None:
                    emit_fin2(prev)
                mx = mxp.tile([P, nch], F32, tag="mx")
                nc.vector.tensor_reduce(
                    mx, trT, axis=mybir.AxisListType.X, op=Alu.max)
                asn = asnp.tile([P, nch, K], FP8, tag="asn")
                nc.vector.scalar_tensor_tensor(
                    out=asn, in0=trT, scalar=1.0,
                    in1=mx[:, :, None].broadcast_to([P, nch, K]),
                    op0=Alu.mult, op1=Alu.is_ge,
                )
                prev = dict(b=b, pairt=pairt, asn=asn)
            emit_agg_fin1(prev)
            emit_fin2(prev)

    _split_multi_waits(nc)
    return nc


def prep_inputs(query_descs, c_centers, imgs=IMGS, npair=NPAIR, ncores=NCORES):
    """Host-side layout prep shared by kernel() and tests."""
    NN = npair * 2 * P
    qd = np.ascontiguousarray(query_descs, dtype=np.float32)
    cc = np.ascontiguousarray(c_centers, dtype=np.float32)
    # normalized descriptors at x64 scale (sweet spot for fp8e4m3); the
    # x64 factors cancel in argmax and under the downstream l2norms
    nrm = np.maximum(np.linalg.norm(qd, axis=-1, keepdims=True), EPS)
    dn8 = (qd / nrm * 64.0).astype(NP_FP8)  # [B', N', D]
    cn = cc / np.maximum(np.linalg.norm(cc, axis=1, keepdims=True), EPS)
    cnT64 = np.ascontiguousarray(cn.T * 64.0).astype(NP_FP8)  # [D, K]
    cnt2 = np.ascontiguousarray(
        cnT64.reshape(CP, 2, P, K).transpose(0, 2, 1, 3)
    ).reshape(CP * P, 2 * K)
    cen16 = cc.astype(NP_BF16)
    ident = np.eye(K, dtype=NP_BF16)
    in_maps = []
    for core in range(ncores):
        sh = dn8[core * imgs:(core + 1) * imgs, :NN]  # [imgs, NN, D]
        nat = np.ascontiguousarray(
            sh.reshape(imgs, npair, 2, P, D).transpose(0, 1, 3, 2, 4)
        ).reshape(imgs * npair * P, 2 * D)
        shT = sh.transpose(0, 2, 1)  # [imgs, D, NN]
        tsp = np.ascontiguousarray(
            shT.reshape(imgs, CP, 2, P, NN).transpose(0, 1, 3, 2, 4)
        ).reshape(imgs * CP * P, 2 * NN)
        in_maps.append({
            "descsn": nat,
            "descst": tsp,
            "cnt2": cnt2,
            "cen": cen16,
            "ident": ident,
        })
    return in_maps


_NC_CACHE = {}


def _get_nc():
    if "nc" not in _NC_CACHE:
        _NC_CACHE["nc"] = build_nc()
    return _NC_CACHE["nc"]


def kernel(query_descs, c_centers):
    in_maps = prep_inputs(query_descs, c_centers)
    nc = _get_nc()
    res = run_bass_kernel_spmd(nc, in_maps, core_ids=list(range(NCORES)))
    out = np.concatenate(
        [res.results[i]["out"] for i in range(NCORES)], axis=0
    )  # [B, K*D]
    return out.astype(np.float32)


# revision 10
# speedup vs baseline: 1.7517x; 1.0184x over previous
"""AnyLoc VLAD (vq_codebook) Trainium2 kernel, 8-core data parallel. v3.

Reference computation (per image, N=1024 patches, K=64 clusters, D=1536):
  descs_n = l2norm(query_descs)                 # row-normalize descriptors
  labels  = argmax_k(descs_n . l2norm(centers)) # hard assignment
  sum_d_k = sum_{n: label=k} descs_n            # per-cluster sum
  un_vlad = sum_d_k - count_k * centers_k
  vlad    = l2norm_rows(un_vlad); flatten; l2norm

Sharding: data-parallel over the batch axis, 4 images per NeuronCore; each
core holds the whole (tiny) codebook; host concatenates the per-core
outputs (no collectives needed).

The kernel is DMA-wire-bound (12.6 MB of fp8 descriptors per core at
~330 GB/s aggregate across the 16 DMA engines), so the structure keeps the
two hardware-DGE queues (sync + scalar) streaming continuously and hides
all compute under them:

  - host pre-normalizes descriptors (fp8 at x64 scale) and ships TWO
    layouts: natural patch-major pair tiles (agg rhs) and a DoubleRow-
    packed transposed layout (sims rhs). Every DMA row is one contiguous
    6 KB per-partition packet.
  - the pipeline runs in HALF-IMAGE slots (512 patches): per slot one tsp
    DMA (sync queue) + one nat DMA (scalar queue), 6 DoubleRow fp8 sims
    matmuls (codebook stationary -> simsT [64,512] psum, 1 bank), an ACT
    copy to bf16, 4 PE transposes back to patch-major (one shared bank;
    `start` only on the first write so a lazy hardware zero cannot wipe
    earlier chunks), one segmented DVE row-max and one is_ge against a
    stride-0 broadcast -> exact 1.0 one-hot fp8.
  - each slot's aggregation (6 DoubleRow matmuls + 2 one-column counts
    matmuls, accumulated per image) is deferred into the NEXT slot's
    stream so TensorE alternates sims_s+1 / agg_s with no idle window.
  - finalize: un_vlad = (-64*counts)*centers + agg in one pass split
    halves across DVE and GPSIMD (reads the agg psum directly, freeing
    banks); row norms via ACT Square+accum; global norm = sqrt(#nonzero
    rows) via a ones-matmul; final scale split ACT/GPSIMD; bf16 output
    (host upcasts) halves the write traffic; out DMAs ride the sync queue
    whose input traffic ends first.
  - PSUM budget exactly 8 banks: 2 sims + 2 transpose + 1 counts + 3 agg.

Toolchain workarounds: this walrus build accepts only one sync wait per
instruction, so Tile's tail drain is re-spread across per-engine drains
and a post-pass hoists surplus waits onto no-op carriers.
"""

import os
import sys

import numpy as np

for _p in ("/opt/trn_rl_repo", "/root/.axon_site/_ro/trn_rl_repo"):
    if os.path.isdir(_p) and _p not in sys.path:
        sys.path.insert(0, _p)

from contextlib import ExitStack

import ml_dtypes
import bass_rust
import concourse.bass as bass
import concourse.tile as tile
from concourse import mybir
from concourse.bass_utils import run_bass_kernel_spmd

B, N, K, D = 32, 1024, 64, 1536
NCORES = 8
IMGS = B // NCORES  # images per core
P = 128
NPAIR = 4   # patch chunk-pairs per image (N = NPAIR*256)
CP = 6      # feature chunk-pairs (D = CP*256)
JJ = D // 512  # agg column blocks
DH = D // 2    # finalize half split
BF16 = mybir.dt.bfloat16
FP8 = mybir.dt.float8e4
F32 = mybir.dt.float32
NP_BF16 = ml_dtypes.bfloat16
NP_FP8 = ml_dtypes.float8_e4m3
Alu = mybir.AluOpType
Act = mybir.ActivationFunctionType
DR = mybir.MatmulPerfMode.DoubleRow
EPS = 1e-12


def _patch_tile_drain():
    """This walrus build only accepts ONE sync wait per instruction; Tile's
    tail drain aggregates every outstanding semaphore wait onto a single
    Drain. Spread the waits across extra per-engine drains (all still
    before the end-of-kernel barrier, so semantics are unchanged)."""
    if getattr(tile.TileContext, "_vlad_drain_patched", False):
        return
    from concourse.vector_clock import ScopedClock

    def patched(self, tick_clock, wait_clock):
        nc = self.nc
        probe = nc.sync.drain()
        wait_clock.add_sem_waits(
            probe.ins, ScopedClock({None: tick_clock.global_clock})
        )
        si = probe.ins.sync_info
        waits = list(si.on_wait) if si is not None else []
        upds = list(si.on_update) if si is not None else []
        probe.ins.sync_info = bass_rust.SyncInfo(on_wait=waits[:1], on_update=upds)
        engines = [nc.scalar, nc.vector, nc.tensor, nc.gpsimd, nc.sync]
        for i, w in enumerate(waits[1:]):
            d = engines[i % len(engines)].drain()
            dsi = d.ins.sync_info
            du = list(dsi.on_update) if dsi is not None else []
            d.ins.sync_info = bass_rust.SyncInfo(on_wait=[w], on_update=du)
        nc.all_engine_barrier()
        popped = nc._tile_sem_poison_stack.pop()
        assert popped is self._sem_poison
        nc.clear_and_free_semaphores(list(self.sems.allocated().values()))

    tile.TileContext._drain_and_barrier = patched
    tile.TileContext._vlad_drain_patched = True


def _split_multi_waits(nc):
    """Walrus here accepts only one sync wait per instruction. Hoist surplus
    waits onto no-op carrier instructions inserted just before, on the same
    engine (safe: same engine executes in order, so all waits still complete
    before the original instruction issues)."""
    n_new = 0
    for _bbname, bassbb in list(nc.bb_map.items()):
        bb = bassbb.bb
        out = []
        changed = False
        for ins in bb.instructions:
            si = getattr(ins, "sync_info", None)
            waits = list(si.on_wait) if si is not None else []
            if len(waits) > 1:
                changed = True
                for w in waits[:-1]:
                    n_new += 1
                    nop = mybir.InstNoOp(
                        name=f"{ins.name}-wsplit{n_new}",
                        sync_info=mybir.SyncInfo(on_wait=[w], on_update=[]),
                        bass_nofuse=True,
                        engine=ins.engine,
                    )
                    nc.register_instruction(nop)
                    out.append(nop)
                ins.sync_info = bass_rust.SyncInfo(
                    on_wait=[waits[-1]], on_update=list(si.on_update)
                )
            out.append(ins)
        if changed:
            bb.instructions = out
    return n_new


def _slot_geom(npair):
    """Half-image slotting: S slots per image, pps chunk-pairs per slot."""
    S = 2 if npair % 2 == 0 and npair >= 2 else 1
    pps = npair // S
    nsl = pps * 2 * P  # patch columns per slot
    return S, pps, nsl


def build_nc(imgs=IMGS, npair=NPAIR):
    """Build the per-core Bass graph. `imgs`/`npair` shrinkable for sim."""
    _patch_tile_drain()
    S, pps, nsl = _slot_geom(npair)
    nch_s = 2 * pps  # 128-patch chunks per slot

    nc = bass.Bass("TRN2", target_bir_lowering=False, debug=False)
    # natural pair tiles: row (slot, p) = 6KB [cp, q, d] flat, where
    # element (cp, q, d) = desc[chunk 2*(slot_pairbase+cp)+q, patch p, d]
    descsn_e = nc.dram_tensor("descsn", [imgs * S * P, pps * 2 * D], FP8,
                              kind="ExternalInput")
    # DoubleRow-packed transpose: row (slot, p) = 6KB [c, q, n] flat with
    # element (c, q, n) = desc[b, slot_n0 + n, 256c+128q+p]
    descst_e = nc.dram_tensor("descst", [imgs * S * P, CP * 2 * nsl], FP8,
                              kind="ExternalInput")
    # codebook, same DoubleRow packing: row (c, p) = [q, k] = cnorm64[k, 256c+128q+p]
    cnt2_e = nc.dram_tensor("cnt2", [CP * P, 2 * K], FP8, kind="ExternalInput")
    cen_e = nc.dram_tensor("cen", [K, D], BF16, kind="ExternalInput")
    ident_e = nc.dram_tensor("ident", [K, K], BF16, kind="ExternalInput")
    out_e = nc.dram_tensor("out", [imgs, K * D], BF16, kind="ExternalOutput")

    with tile.TileContext(nc) as tc:
        with ExitStack() as ctx:
            consts = ctx.enter_context(tc.tile_pool(name="consts", bufs=1))
            tspp = ctx.enter_context(tc.tile_pool(name="tspp", bufs=4))
            natp = ctx.enter_context(tc.tile_pool(name="natp", bufs=4))
            simsbp = ctx.enter_context(tc.tile_pool(name="simsbp", bufs=2))
            asnp = ctx.enter_context(tc.tile_pool(name="asnp", bufs=2))
            mxp = ctx.enter_context(tc.tile_pool(name="mxp", bufs=2))
            sqp = ctx.enter_context(tc.tile_pool(name="sqp", bufs=2))
            vfinp = ctx.enter_context(tc.tile_pool(name="vfinp", bufs=2))
            finp = ctx.enter_context(tc.tile_pool(name="finp", bufs=16))
            simsps = ctx.enter_context(
                tc.tile_pool(name="simsps", bufs=2, space="PSUM"))
            transps = ctx.enter_context(
                tc.tile_pool(name="transps", bufs=2, space="PSUM"))
            cntps = ctx.enter_context(
                tc.tile_pool(name="cntps", bufs=1, space="PSUM"))
            aggps = ctx.enter_context(
                tc.tile_pool(name="aggps", bufs=1, space="PSUM"))

            cnt_sb = consts.tile([P, CP, 2, K], FP8)
            nc.sync.dma_start(
                out=cnt_sb,
                in_=cnt2_e.ap().rearrange("(c p) (q k) -> p c q k", c=CP, q=2),
            )
            ident_sb = consts.tile([K, K], BF16)
            nc.sync.dma_start(out=ident_sb, in_=ident_e.ap())
            onesc = consts.tile([P, 2, 1], FP8)
            nc.vector.memset(onesc, 1.0)
            # finalize-only consts load lazily in slot 0's shadow
            late = {}

            def _late():
                if not late:
                    cen_sb = consts.tile([K, D], BF16)
                    nc.scalar.dma_start(out=cen_sb, in_=cen_e.ap())
                    onesg = consts.tile([K, K], BF16)
                    nc.vector.memset(onesg, 1.0)
                    late.update(cen_sb=cen_sb, onesg=onesg)
                return late

            img_agg = {}

            def emit_agg(s):
                """Aggregate slot s's one-hot (deferred one slot)."""
                b, sj = s["b"], s["sj"]
                if sj == 0:
                    img_agg[b] = dict(
                        agg=aggps.tile([K, JJ, 512], F32, tag="agg", name="agg"),
                        counts=cntps.tile([K, 1], F32, tag="cnt", name="counts"),
                    )
                ia = img_agg[b]
                for cp in range(pps):
                    for jj in range(JJ):
                        nc.tensor.matmul(
                            ia["agg"][:, jj, :],
                            lhsT=s["asn"][:, 2 * cp:2 * cp + 2, :],
                            rhs=s["nat"][:, cp, :, jj * 512:(jj + 1) * 512],
                            start=(sj == 0 and cp == 0), stop=False,
                            perf_mode=DR, skip_group_check=True,
                        )
                for cp in range(pps):
                    nc.tensor.matmul(
                        ia["counts"], lhsT=s["asn"][:, 2 * cp:2 * cp + 2, :],
                        rhs=onesc,
                        start=(sj == 0 and cp == 0),
                        stop=(sj == S - 1 and cp == pps - 1),
                        perf_mode=DR, skip_group_check=True,
                    )

            def emit_fin1(b):
                """Image finalize part 1: un_vlad + row norm accumulation.
                GPSIMD cannot touch PSUM, so the -64*counts*centers term is
                folded into the agg psum by a diagonal-lhsT matmul (TensorE
                has slack in this DMA-bound regime) and the norm accumulates
                on ACT straight from PSUM."""
                lc = _late()
                ia = img_agg.pop(b)
                negc = finp.tile([K, 1], F32, tag="negc")
                nc.vector.tensor_scalar_mul(negc, ia["counts"], -64.0)
                diag = finp.tile([K, K], BF16, tag="diag")
                nc.vector.tensor_scalar(
                    diag, ident_sb, scalar1=negc, scalar2=None, op0=Alu.mult)
                for jj in range(JJ):
                    nc.tensor.matmul(
                        ia["agg"][:, jj, :], lhsT=diag,
                        rhs=lc["cen_sb"][:, jj * 512:(jj + 1) * 512],
                        start=False, stop=(jj == JJ - 1),
                        skip_group_check=True,
                    )
                aggflat = ia["agg"].rearrange("k a b -> k (a b)")
                sq = sqp.tile([K, D], FP8, tag="sq")
                r2 = finp.tile([K, 1], F32, tag="r2")
                nc.scalar.activation(sq, aggflat, Act.Square, accum_out=r2)
                u = finp.tile([K, 1], F32, tag="u")
                nc.scalar.sqrt(u, r2)
                return dict(b=b, agg=ia["agg"], u=u)

            def emit_fin2(f):
                """Image finalize part 2: scales + output DMA."""
                lc = _late()
                um = finp.tile([K, 1], F32, tag="um")
                nc.vector.tensor_scalar_max(um, f["u"], EPS)
                invu = finp.tile([K, 1], F32, tag="invu")
                nc.vector.reciprocal(invu, um)
                sgate = finp.tile([K, 1], BF16, tag="sgate")
                nc.vector.tensor_scalar(
                    sgate, f["u"], scalar1=1e30, scalar2=1.0,
                    op0=Alu.mult, op1=Alu.min,
                )
                g_ps = cntps.tile([K, 1], F32, tag="cnt")
                nc.tensor.matmul(g_ps, lhsT=lc["onesg"], rhs=sgate,
                                 start=True, stop=True, skip_group_check=True)
                sg = finp.tile([K, 1], F32, tag="sg")
                nc.scalar.sqrt(sg, g_ps)
                ginv = finp.tile([K, 1], F32, tag="ginv")
                nc.vector.reciprocal(ginv, sg)
                tot = finp.tile([K, 1], F32, tag="tot")
                nc.vector.tensor_mul(tot, invu, ginv)
                vfin = vfinp.tile([K, D], BF16, tag="vfin")
                aggflat = f["agg"].rearrange("k a b -> k (a b)")
                nc.scalar.mul(vfin[:, 0:DH], aggflat[:, 0:DH], tot)
                nc.vector.tensor_scalar_mul(vfin[:, DH:D], aggflat[:, DH:D], tot)
                out_kd = out_e.ap()[f["b"]].rearrange("(k d) -> k d", k=K)
                nc.sync.dma_start(out=out_kd, in_=vfin)

            prev = None       # last slot's state (agg deferred one slot)
            fin1_pending = None  # fin1 emitted, fin2 pending
            for t in range(imgs * S):
                b, sj = divmod(t, S)
                tsp = tspp.tile([P, CP, 2, nsl], FP8, tag="tsp")
                nc.sync.dma_start(
                    out=tsp,
                    in_=descst_e.ap()[t * P:(t + 1) * P, :]
                    .rearrange("p (c q n) -> p c q n", c=CP, q=2),
                )
                nat = natp.tile([P, pps, 2, D], FP8, tag="nat")
                nc.scalar.dma_start(
                    out=nat,
                    in_=descsn_e.ap()[t * P:(t + 1) * P, :]
                    .rearrange("p (c q d) -> p c q d", c=pps, q=2),
                )
                # sims^T: codebook-stationary DoubleRow accumulation
                simsT = simsps.tile([K, nsl], F32, tag="sims")
                for c in range(CP):
                    nc.tensor.matmul(
                        simsT, lhsT=cnt_sb[:, c], rhs=tsp[:, c],
                        start=(c == 0), stop=(c == CP - 1),
                        perf_mode=DR, skip_group_check=True,
                    )
                # deferred aggregation of the previous slot keeps TensorE
                # busy while this slot's sims copy/transpose round-trips
                if prev is not None:
                    emit_agg(prev)
                    if prev["sj"] == S - 1:
                        fin1_pending = emit_fin1(prev["b"])
                simsSb = simsbp.tile([K, nsl], BF16, tag="simsb")
                nc.scalar.mul(simsSb, simsT, 1.0)
                # transpose back to patch-major; ONE shared bank, start only
                # on the first write (re-marking the zero region would let a
                # lazy hardware zero wipe earlier chunks)
                trT = transps.tile([P, nch_s, K], BF16, tag="tr")
                for ch in range(nch_s):
                    nc.tensor.matmul(
                        trT[:, ch, :], lhsT=simsSb[:, ch * P:(ch + 1) * P],
                        rhs=ident_sb, is_transpose=True,
                        start=(ch == 0), stop=(ch == nch_s - 1),
                        skip_group_check=True,
                    )
                mx = mxp.tile([P, nch_s], F32, tag="mx")
                nc.vector.tensor_reduce(
                    mx, trT, axis=mybir.AxisListType.X, op=Alu.max)
                asn = asnp.tile([P, nch_s, K], FP8, tag="asn")
                nc.vector.scalar_tensor_tensor(
                    out=asn, in0=trT, scalar=1.0,
                    in1=mx[:, :, None].broadcast_to([P, nch_s, K]),
                    op0=Alu.mult, op1=Alu.is_ge,
                )
                # fin2 of the image whose fin1 opened this stream: by now
                # its ACT sqrt round-trip has completed, so the DVE chain
                # here doesn't stall, and the agg banks free before the
                # next image's aggregation rotates into them
                if fin1_pending is not None:
                    emit_fin2(fin1_pending)
                    fin1_pending = None
                prev = dict(b=b, sj=sj, nat=nat, asn=asn)
            emit_agg(prev)
            emit_fin2(emit_fin1(prev["b"]))

    _split_multi_waits(nc)
    return nc


def prep_inputs(query_descs, c_centers, imgs=IMGS, npair=NPAIR, ncores=NCORES):
    """Host-side layout prep shared by kernel() and tests."""
    S, pps, nsl = _slot_geom(npair)
    NN = npair * 2 * P
    qd = np.ascontiguousarray(query_descs, dtype=np.float32)
    cc = np.ascontiguousarray(c_centers, dtype=np.float32)
    # normalized descriptors at x64 scale (sweet spot for fp8e4m3); the
    # x64 factors cancel in argmax and under the downstream l2norms
    nrm = np.maximum(np.linalg.norm(qd, axis=-1, keepdims=True), EPS)
    dn8 = (qd / nrm * 64.0).astype(NP_FP8)  # [B', N', D]
    cn = cc / np.maximum(np.linalg.norm(cc, axis=1, keepdims=True), EPS)
    cnT64 = np.ascontiguousarray(cn.T * 64.0).astype(NP_FP8)  # [D, K]
    cnt2 = np.ascontiguousarray(
        cnT64.reshape(CP, 2, P, K).transpose(0, 2, 1, 3)
    ).reshape(CP * P, 2 * K)
    cen16 = cc.astype(NP_BF16)
    ident = np.eye(K, dtype=NP_BF16)
    in_maps = []
    for core in range(ncores):
        sh = dn8[core * imgs:(core + 1) * imgs, :NN]  # [imgs, NN, D]
        # nat row (b, s, p) = [cp, q, d] flat (6KB contiguous per packet)
        nat = np.ascontiguousarray(
            sh.reshape(imgs, S, pps, 2, P, D).transpose(0, 1, 4, 2, 3, 5)
        ).reshape(imgs * S * P, pps * 2 * D)
        # tsp row (b, s, p) = [c, q, n] flat with (c,q,n) = desc[b, n0+n, 256c+128q+p]
        shT = sh.transpose(0, 2, 1)  # [imgs, D, NN]
        tsp = np.ascontiguousarray(
            shT.reshape(imgs, CP, 2, P, S, nsl).transpose(0, 4, 3, 1, 2, 5)
        ).reshape(imgs * S * P, CP * 2 * nsl)
        in_maps.append({
            "descsn": nat,
            "descst": tsp,
            "cnt2": cnt2,
            "cen": cen16,
            "ident": ident,
        })
    return in_maps


_NC_CACHE = {}


def _get_nc():
    if "nc" not in _NC_CACHE:
        _NC_CACHE["nc"] = build_nc()
    return _NC_CACHE["nc"]


def kernel(query_descs, c_centers):
    in_maps = prep_inputs(query_descs, c_centers)
    nc = _get_nc()
    res = run_bass_kernel_spmd(nc, in_maps, core_ids=list(range(NCORES)))
    out = np.concatenate(
        [res.results[i]["out"] for i in range(NCORES)], axis=0
    )  # [B, K*D] bf16
    return out.astype(np.float32)


# revision 15
# speedup vs baseline: 1.8170x; 1.0373x over previous
"""AnyLoc VLAD (vq_codebook) Trainium2 kernel, 8-core data parallel. v3.

Reference computation (per image, N=1024 patches, K=64 clusters, D=1536):
  descs_n = l2norm(query_descs)                 # row-normalize descriptors
  labels  = argmax_k(descs_n . l2norm(centers)) # hard assignment
  sum_d_k = sum_{n: label=k} descs_n            # per-cluster sum
  un_vlad = sum_d_k - count_k * centers_k
  vlad    = l2norm_rows(un_vlad); flatten; l2norm

Sharding: data-parallel over the batch axis, 4 images per NeuronCore; each
core holds the whole (tiny) codebook; host concatenates the per-core
outputs (no collectives needed).

The kernel is DMA-wire-bound (12.6 MB of fp8 descriptors per core at
~330 GB/s aggregate across the 16 DMA engines), so the structure keeps the
two hardware-DGE queues (sync + scalar) streaming continuously and hides
all compute under them:

  - host pre-normalizes descriptors (fp8 at x64 scale) and ships TWO
    layouts: natural patch-major pair tiles (agg rhs) and a DoubleRow-
    packed transposed layout (sims rhs). Every DMA row is one contiguous
    6 KB per-partition packet.
  - the pipeline runs in HALF-IMAGE slots (512 patches): per slot one tsp
    DMA (sync queue) + one nat DMA (scalar queue), 6 DoubleRow fp8 sims
    matmuls (codebook stationary -> simsT [64,512] psum, 1 bank), an ACT
    copy to bf16, 4 PE transposes back to patch-major (one shared bank;
    `start` only on the first write so a lazy hardware zero cannot wipe
    earlier chunks), one segmented DVE row-max and one is_ge against a
    stride-0 broadcast -> exact 1.0 one-hot fp8.
  - each slot's aggregation (6 DoubleRow matmuls + 2 one-column counts
    matmuls, accumulated per image) is deferred into the NEXT slot's
    stream so TensorE alternates sims_s+1 / agg_s with no idle window.
  - finalize: un_vlad = (-64*counts)*centers + agg in one pass split
    halves across DVE and GPSIMD (reads the agg psum directly, freeing
    banks); row norms via ACT Square+accum; global norm = sqrt(#nonzero
    rows) via a ones-matmul; final scale split ACT/GPSIMD; bf16 output
    (host upcasts) halves the write traffic; out DMAs ride the sync queue
    whose input traffic ends first.
  - PSUM budget exactly 8 banks: 2 sims + 2 transpose + 1 counts + 3 agg.

Toolchain workarounds: this walrus build accepts only one sync wait per
instruction, so Tile's tail drain is re-spread across per-engine drains
and a post-pass hoists surplus waits onto no-op carriers.
"""

import os
import sys

import numpy as np

for _p in ("/opt/trn_rl_repo", "/root/.axon_site/_ro/trn_rl_repo"):
    if os.path.isdir(_p) and _p not in sys.path:
        sys.path.insert(0, _p)

from contextlib import ExitStack

import ml_dtypes
import bass_rust
import concourse.bass as bass
import concourse.tile as tile
from concourse import mybir
from concourse.bass_utils import run_bass_kernel_spmd

B, N, K, D = 32, 1024, 64, 1536
NCORES = 8
IMGS = B // NCORES  # images per core
P = 128
NPAIR = 4   # patch chunk-pairs per image (N = NPAIR*256)
CP = 6      # feature chunk-pairs (D = CP*256)
JJ = D // 512  # agg column blocks
DH = D // 2    # finalize half split
BF16 = mybir.dt.bfloat16
FP8 = mybir.dt.float8e4
F32 = mybir.dt.float32
NP_BF16 = ml_dtypes.bfloat16
NP_FP8 = ml_dtypes.float8_e4m3
Alu = mybir.AluOpType
Act = mybir.ActivationFunctionType
DR = mybir.MatmulPerfMode.DoubleRow
EPS = 1e-12


def _patch_tile_drain():
    """This walrus build only accepts ONE sync wait per instruction; Tile's
    tail drain aggregates every outstanding semaphore wait onto a single
    Drain. Spread the waits across extra per-engine drains (all still
    before the end-of-kernel barrier, so semantics are unchanged)."""
    if getattr(tile.TileContext, "_vlad_drain_patched", False):
        return
    from concourse.vector_clock import ScopedClock

    def patched(self, tick_clock, wait_clock):
        nc = self.nc
        probe = nc.sync.drain()
        wait_clock.add_sem_waits(
            probe.ins, ScopedClock({None: tick_clock.global_clock})
        )
        si = probe.ins.sync_info
        waits = list(si.on_wait) if si is not None else []
        upds = list(si.on_update) if si is not None else []
        probe.ins.sync_info = bass_rust.SyncInfo(on_wait=waits[:1], on_update=upds)
        engines = [nc.scalar, nc.vector, nc.tensor, nc.gpsimd, nc.sync]
        for i, w in enumerate(waits[1:]):
            d = engines[i % len(engines)].drain()
            dsi = d.ins.sync_info
            du = list(dsi.on_update) if dsi is not None else []
            d.ins.sync_info = bass_rust.SyncInfo(on_wait=[w], on_update=du)
        nc.all_engine_barrier()
        popped = nc._tile_sem_poison_stack.pop()
        assert popped is self._sem_poison
        nc.clear_and_free_semaphores(list(self.sems.allocated().values()))

    tile.TileContext._drain_and_barrier = patched
    tile.TileContext._vlad_drain_patched = True


def _split_multi_waits(nc):
    """Walrus here accepts only one sync wait per instruction. Hoist surplus
    waits onto no-op carrier instructions inserted just before, on the same
    engine (safe: same engine executes in order, so all waits still complete
    before the original instruction issues)."""
    n_new = 0
    for _bbname, bassbb in list(nc.bb_map.items()):
        bb = bassbb.bb
        out = []
        changed = False
        for ins in bb.instructions:
            si = getattr(ins, "sync_info", None)
            waits = list(si.on_wait) if si is not None else []
            if len(waits) > 1:
                changed = True
                for w in waits[:-1]:
                    n_new += 1
                    nop = mybir.InstNoOp(
                        name=f"{ins.name}-wsplit{n_new}",
                        sync_info=mybir.SyncInfo(on_wait=[w], on_update=[]),
                        bass_nofuse=True,
                        engine=ins.engine,
                    )
                    nc.register_instruction(nop)
                    out.append(nop)
                ins.sync_info = bass_rust.SyncInfo(
                    on_wait=[waits[-1]], on_update=list(si.on_update)
                )
            out.append(ins)
        if changed:
            bb.instructions = out
    return n_new


def _slot_geom(npair):
    """Half-image slotting: S slots per image, pps chunk-pairs per slot."""
    S = 2 if npair % 2 == 0 and npair >= 2 else 1
    pps = npair // S
    nsl = pps * 2 * P  # patch columns per slot
    return S, pps, nsl


def build_nc(imgs=IMGS, npair=NPAIR):
    """Build the per-core Bass graph. `imgs`/`npair` shrinkable for sim."""
    _patch_tile_drain()
    S, pps, nsl = _slot_geom(npair)
    nch_s = 2 * pps  # 128-patch chunks per slot

    nc = bass.Bass("TRN2", target_bir_lowering=False, debug=False)
    # natural pair tiles: row (slot, p) = 6KB [cp, q, d] flat, where
    # element (cp, q, d) = desc[chunk 2*(slot_pairbase+cp)+q, patch p, d]
    descsn_e = nc.dram_tensor("descsn", [imgs * S * P, pps * 2 * D], FP8,
                              kind="ExternalInput")
    # DoubleRow-packed transpose: row (slot, p) = 6KB [c, q, n] flat with
    # element (c, q, n) = desc[b, slot_n0 + n, 256c+128q+p]
    descst_e = nc.dram_tensor("descst", [imgs * S * P, CP * 2 * nsl], FP8,
                              kind="ExternalInput")
    # codebook, same DoubleRow packing: row (c, p) = [q, k] = cnorm64[k, 256c+128q+p]
    cnt2_e = nc.dram_tensor("cnt2", [CP * P, 2 * K], FP8, kind="ExternalInput")
    cen_e = nc.dram_tensor("cen", [K, D], BF16, kind="ExternalInput")
    ident_e = nc.dram_tensor("ident", [K, K], BF16, kind="ExternalInput")
    out_e = nc.dram_tensor("out", [imgs, K * D], BF16, kind="ExternalOutput")

    with tile.TileContext(nc) as tc:
        with ExitStack() as ctx:
            consts = ctx.enter_context(tc.tile_pool(name="consts", bufs=1))
            tspp = ctx.enter_context(tc.tile_pool(name="tspp", bufs=6))
            natp = ctx.enter_context(tc.tile_pool(name="natp", bufs=6))
            simsbp = ctx.enter_context(tc.tile_pool(name="simsbp", bufs=2))
            asnp = ctx.enter_context(tc.tile_pool(name="asnp", bufs=2))
            mxp = ctx.enter_context(tc.tile_pool(name="mxp", bufs=2))
            uvp = ctx.enter_context(tc.tile_pool(name="uvp", bufs=2))
            sqp = ctx.enter_context(tc.tile_pool(name="sqp", bufs=2))
            vfinp = ctx.enter_context(tc.tile_pool(name="vfinp", bufs=2))
            finp = ctx.enter_context(tc.tile_pool(name="finp", bufs=16))
            simsps = ctx.enter_context(
                tc.tile_pool(name="simsps", bufs=2, space="PSUM"))
            transps = ctx.enter_context(
                tc.tile_pool(name="transps", bufs=2, space="PSUM"))
            cntps = ctx.enter_context(
                tc.tile_pool(name="cntps", bufs=1, space="PSUM"))
            aggps = ctx.enter_context(
                tc.tile_pool(name="aggps", bufs=1, space="PSUM"))

            cnt_sb = consts.tile([P, CP, 2, K], FP8)
            nc.sync.dma_start(
                out=cnt_sb,
                in_=cnt2_e.ap().rearrange("(c p) (q k) -> p c q k", c=CP, q=2),
            )
            ident_sb = consts.tile([K, K], BF16)
            nc.sync.dma_start(out=ident_sb, in_=ident_e.ap())
            onesc = consts.tile([P, 2, 1], FP8)
            nc.vector.memset(onesc, 1.0)
            # finalize-only consts load lazily in slot 0's shadow
            late = {}

            def _late():
                if not late:
                    cen_sb = consts.tile([K, D], BF16)
                    nc.scalar.dma_start(out=cen_sb, in_=cen_e.ap())
                    onesg = consts.tile([K, K], BF16)
                    nc.vector.memset(onesg, 1.0)
                    late.update(cen_sb=cen_sb, onesg=onesg)
                return late

            img_agg = {}

            def emit_agg(s):
                """Aggregate slot s's one-hot (deferred one slot)."""
                b, sj = s["b"], s["sj"]
                if sj == 0:
                    img_agg[b] = dict(
                        agg=aggps.tile([K, JJ, 512], F32, tag="agg", name="agg"),
                        counts=cntps.tile([K, 1], F32, tag="cnt", name="counts"),
                    )
                ia = img_agg[b]
                for cp in range(pps):
                    for jj in range(JJ):
                        nc.tensor.matmul(
                            ia["agg"][:, jj, :],
                            lhsT=s["asn"][:, 2 * cp:2 * cp + 2, :],
                            rhs=s["nat"][:, cp, :, jj * 512:(jj + 1) * 512],
                            start=(sj == 0 and cp == 0), stop=False,
                            perf_mode=DR, skip_group_check=True,
                        )
                for cp in range(pps):
                    nc.tensor.matmul(
                        ia["counts"], lhsT=s["asn"][:, 2 * cp:2 * cp + 2, :],
                        rhs=onesc,
                        start=(sj == 0 and cp == 0),
                        stop=(sj == S - 1 and cp == pps - 1),
                        perf_mode=DR, skip_group_check=True,
                    )

            def emit_fin1(b):
                """Image finalize part 1: un_vlad + row norm accumulation.
                GPSIMD cannot touch PSUM, so the -64*counts*centers term is
                folded into the agg psum by a diagonal-lhsT matmul (TensorE
                has slack in this DMA-bound regime). un_vlad is then copied
                to SBUF in ACT/DVE halves, releasing the agg banks early so
                the next image's aggregation never stalls on this finalize;
                the row-norm accumulation also runs as parallel halves."""
                lc = _late()
                ia = img_agg.pop(b)
                negc = finp.tile([K, 1], F32, tag="negc")
                nc.vector.tensor_scalar_mul(negc, ia["counts"], -64.0)
                diag = finp.tile([K, K], BF16, tag="diag")
                nc.vector.tensor_scalar(
                    diag, ident_sb, scalar1=negc, scalar2=None, op0=Alu.mult)
                for jj in range(JJ):
                    nc.tensor.matmul(
                        ia["agg"][:, jj, :], lhsT=diag,
                        rhs=lc["cen_sb"][:, jj * 512:(jj + 1) * 512],
                        start=False, stop=(jj == JJ - 1),
                        skip_group_check=True,
                    )
                aggflat = ia["agg"].rearrange("k a b -> k (a b)")
                uv = uvp.tile([K, D], F32, tag="uv")
                nc.scalar.mul(uv[:, 0:DH], aggflat[:, 0:DH], 1.0)
                nc.vector.tensor_scalar_mul(uv[:, DH:D], aggflat[:, DH:D], 1.0)
                sq = sqp.tile([K, D], FP8, tag="sq")
                r2a = finp.tile([K, 1], F32, tag="r2a")
                nc.scalar.activation(sq[:, 0:DH], uv[:, 0:DH], Act.Square,
                                     accum_out=r2a)
                r2b = finp.tile([K, 1], F32, tag="r2b")
                nc.vector.scalar_tensor_tensor(
                    out=sq[:, DH:D], in0=uv[:, DH:D], scalar=1.0,
                    in1=uv[:, DH:D], op0=Alu.mult, op1=Alu.mult,
                    accum_out=r2b,
                )
                r2 = finp.tile([K, 1], F32, tag="r2")
                nc.vector.tensor_tensor(r2, r2a, r2b, op=Alu.add)
                u = finp.tile([K, 1], F32, tag="u")
                nc.scalar.sqrt(u, r2)
                return dict(b=b, uv=uv, u=u)

            def emit_fin2(f):
                """Image finalize part 2: scales + output DMA."""
                lc = _late()
                um = finp.tile([K, 1], F32, tag="um")
                nc.vector.tensor_scalar_max(um, f["u"], EPS)
                invu = finp.tile([K, 1], F32, tag="invu")
                nc.vector.reciprocal(invu, um)
                sgate = finp.tile([K, 1], BF16, tag="sgate")
                nc.vector.tensor_scalar(
                    sgate, f["u"], scalar1=1e30, scalar2=1.0,
                    op0=Alu.mult, op1=Alu.min,
                )
                g_ps = transps.tile([K, 1], F32, tag="tr", name="g_ps")
                nc.tensor.matmul(g_ps, lhsT=lc["onesg"], rhs=sgate,
                                 start=True, stop=True, skip_group_check=True)
                sg = finp.tile([K, 1], F32, tag="sg")
                nc.scalar.sqrt(sg, g_ps)
                ginv = finp.tile([K, 1], F32, tag="ginv")
                nc.vector.reciprocal(ginv, sg)
                tot = finp.tile([K, 1], F32, tag="tot")
                nc.vector.tensor_mul(tot, invu, ginv)
                # final scale from the SBUF un_vlad, halves on ACT/DVE
                vfin = vfinp.tile([K, D], BF16, tag="vfin")
                nc.scalar.mul(vfin[:, 0:DH], f["uv"][:, 0:DH], tot)
                nc.vector.tensor_scalar_mul(
                    vfin[:, DH:D], f["uv"][:, DH:D], tot)
                out_kd = out_e.ap()[f["b"]].rearrange("(k d) -> k d", k=K)
                nc.sync.dma_start(out=out_kd, in_=vfin)

            prev = None       # last slot's state (agg deferred one slot)
            fin1_pending = None  # fin1 emitted, fin2 pending
            for t in range(imgs * S):
                b, sj = divmod(t, S)
                tsp = tspp.tile([P, CP, 2, nsl], FP8, tag="tsp")
                nc.sync.dma_start(
                    out=tsp,
                    in_=descst_e.ap()[t * P:(t + 1) * P, :]
                    .rearrange("p (c q n) -> p c q n", c=CP, q=2),
                )
                nat = natp.tile([P, pps, 2, D], FP8, tag="nat")
                nc.scalar.dma_start(
                    out=nat,
                    in_=descsn_e.ap()[t * P:(t + 1) * P, :]
                    .rearrange("p (c q d) -> p c q d", c=pps, q=2),
                )
                # sims^T: codebook-stationary DoubleRow accumulation
                simsT = simsps.tile([K, nsl], F32, tag="sims")
                for c in range(CP):
                    nc.tensor.matmul(
                        simsT, lhsT=cnt_sb[:, c], rhs=tsp[:, c],
                        start=(c == 0), stop=(c == CP - 1),
                        perf_mode=DR, skip_group_check=True,
                    )
                # deferred aggregation of the previous slot keeps TensorE
                # busy while this slot's sims copy/transpose round-trips
                if prev is not None:
                    emit_agg(prev)
                    if prev["sj"] == S - 1:
                        fin1_pending = emit_fin1(prev["b"])
                simsSb = simsbp.tile([K, nsl], BF16, tag="simsb")
                nc.scalar.mul(simsSb, simsT, 1.0)
                # transpose back to patch-major; ONE shared bank, start only
                # on the first write (re-marking the zero region would let a
                # lazy hardware zero wipe earlier chunks)
                trT = transps.tile([P, nch_s, K], BF16, tag="tr")
                for ch in range(nch_s):
                    nc.tensor.matmul(
                        trT[:, ch, :], lhsT=simsSb[:, ch * P:(ch + 1) * P],
                        rhs=ident_sb, is_transpose=True,
                        start=(ch == 0), stop=(ch == nch_s - 1),
                        skip_group_check=True,
                    )
                mx = mxp.tile([P, nch_s], F32, tag="mx")
                nc.vector.tensor_reduce(
                    mx, trT, axis=mybir.AxisListType.X, op=Alu.max)
                asn = asnp.tile([P, nch_s, K], FP8, tag="asn")
                nc.vector.scalar_tensor_tensor(
                    out=asn, in0=trT, scalar=1.0,
                    in1=mx[:, :, None].broadcast_to([P, nch_s, K]),
                    op0=Alu.mult, op1=Alu.is_ge,
                )
                # fin2 of the image whose fin1 opened this stream: by now
                # its ACT sqrt round-trip has completed, so the DVE chain
                # here doesn't stall, and the agg banks free before the
                # next image's aggregation rotates into them
                if fin1_pending is not None:
                    emit_fin2(fin1_pending)
                    fin1_pending = None
                prev = dict(b=b, sj=sj, nat=nat, asn=asn)
            emit_agg(prev)
            emit_fin2(emit_fin1(prev["b"]))

    _split_multi_waits(nc)
    return nc


def prep_inputs(query_descs, c_centers, imgs=IMGS, npair=NPAIR, ncores=NCORES):
    """Host-side layout prep shared by kernel() and tests."""
    S, pps, nsl = _slot_geom(npair)
    NN = npair * 2 * P
    qd = np.ascontiguousarray(query_descs, dtype=np.float32)
    cc = np.ascontiguousarray(c_centers, dtype=np.float32)
    # normalized descriptors at x64 scale (sweet spot for fp8e4m3); the
    # x64 factors cancel in argmax and under the downstream l2norms
    nrm = np.maximum(np.linalg.norm(qd, axis=-1, keepdims=True), EPS)
    dn8 = (qd / nrm * 64.0).astype(NP_FP8)  # [B', N', D]
    cn = cc / np.maximum(np.linalg.norm(cc, axis=1, keepdims=True), EPS)
    cnT64 = np.ascontiguousarray(cn.T * 64.0).astype(NP_FP8)  # [D, K]
    cnt2 = np.ascontiguousarray(
        cnT64.reshape(CP, 2, P, K).transpose(0, 2, 1, 3)
    ).reshape(CP * P, 2 * K)
    cen16 = cc.astype(NP_BF16)
    ident = np.eye(K, dtype=NP_BF16)
    in_maps = []
    for core in range(ncores):
        sh = dn8[core * imgs:(core + 1) * imgs, :NN]  # [imgs, NN, D]
        # nat row (b, s, p) = [cp, q, d] flat (6KB contiguous per packet)
        nat = np.ascontiguousarray(
            sh.reshape(imgs, S, pps, 2, P, D).transpose(0, 1, 4, 2, 3, 5)
        ).reshape(imgs * S * P, pps * 2 * D)
        # tsp row (b, s, p) = [c, q, n] flat with (c,q,n) = desc[b, n0+n, 256c+128q+p]
        shT = sh.transpose(0, 2, 1)  # [imgs, D, NN]
        tsp = np.ascontiguousarray(
            shT.reshape(imgs, CP, 2, P, S, nsl).transpose(0, 4, 3, 1, 2, 5)
        ).reshape(imgs * S * P, CP * 2 * nsl)
        in_maps.append({
            "descsn": nat,
            "descst": tsp,
            "cnt2": cnt2,
            "cen": cen16,
            "ident": ident,
        })
    return in_maps


_NC_CACHE = {}


def _get_nc():
    if "nc" not in _NC_CACHE:
        _NC_CACHE["nc"] = build_nc()
    return _NC_CACHE["nc"]


def kernel(query_descs, c_centers):
    in_maps = prep_inputs(query_descs, c_centers)
    nc = _get_nc()
    res = run_bass_kernel_spmd(nc, in_maps, core_ids=list(range(NCORES)))
    out = np.concatenate(
        [res.results[i]["out"] for i in range(NCORES)], axis=0
    )  # [B, K*D] bf16
    return out.astype(np.float32)
